# revision 1
# baseline (speedup 1.0000x reference)
"""GroupedQueryAttention Trainium2 kernel (8 NeuronCores).

Sharding: core i handles (batch b = i//4, KV group g = i%4): its 4 query
heads + 1 KV group, full sequence. Each core computes a partial output
(attn_heads @ Wo rows for its heads); host sums the 4 partials per batch.

Layout strategy (per core):
  - everything transposed: qT/kT [d, t] computed with W-stationary matmuls
  - RoPE: host permutes W rows to half-split layout; swap-half via a
    permutation matmul on PE; cos/sin tables applied on DVE
  - attention: scoresT [s, tq] = kT_tile^T @ qT (stationary kT tile),
    exp on ACT (no max subtraction -- scores are bounded by construction),
    denominators via ones-vector matmul, PV with v[s,d]-stationary
    accumulation -> outT [d, tq], normalization by broadcast reciprocal.
  - out projection: attn_flatT chunks stationary, Wo.T moving.
All matmuls run as float32r (full-rate fp32 PE mode).
"""

import numpy as np
from contextlib import ExitStack

import concourse.bass as bass
import concourse.bacc as bacc
import concourse.tile as tile
import concourse.mybir as mybir
from concourse.bass_utils import run_bass_kernel_spmd

# problem shape (hardcoded per contract)
B, T, E = 2, 2048, 2048
NH, NG, HD = 16, 4, 128
HPG = NH // NG          # 4 heads per group = per core
NE = E // 128           # 16 contraction chunks
TB = 512                # tq / t block
NTB = T // TB           # 4
NST = T // 128          # 16 s-tiles
F32 = mybir.dt.float32
F32R = mybir.dt.float32r
EXP = mybir.ActivationFunctionType.Exp

N_CORES = 8


def _r(ap):
    return ap.bitcast(F32R)


def build_body(tc, out_ap, ins):
    """ins: dict name -> dram AP. out_ap: [T, E] dram AP."""
    nc = tc.nc
    ctx = ExitStack()
    with ctx:
        ctx.enter_context(nc.allow_low_precision(
            reason="fp32r rounding on matmul inputs is intended"))
        # ---- constant / persistent SBUF ----
        const = ctx.enter_context(tc.tile_pool(name="const", bufs=1))
        cs2 = const.tile([128, T], F32, tag="cs2", name="cs2")
        snpm = const.tile([128, T], F32, tag="snpm", name="snpm")
        tri = const.tile([128, 128], F32, tag="tri", name="tri")
        swp = const.tile([128, 128], F32R, tag="swp", name="swp")
        iden = const.tile([128, 128], F32, tag="iden", name="iden")
        ones = const.tile([128, 128], F32R, tag="ones", name="ones")
        zer = const.tile([128, TB], F32, tag="zer", name="zer")

        persist = ctx.enter_context(tc.tile_pool(name="persist", bufs=1))
        qrot = [persist.tile([128, T], F32, tag=f"qrot{h}", name=f"qrot{h}") for h in range(HPG)]
        krot = persist.tile([128, T], F32, tag="krot", name="krot")
        vsd = persist.tile([128, T], F32, tag="vsd", name="vsd")
        aout = qrot  # attn output overwrites qrot block-by-block (dead after scores)

        # ---- weights (packed into single wide tiles, col block = e-chunk) ----
        wpool = ctx.enter_context(tc.tile_pool(name="weights", bufs=1))
        wq_t = wpool.tile([128, NE * 512], F32R, tag="wbig", name="wq")    # block e: [128, 4*128]
        wk_t = wpool.tile([128, NE * 128], F32R, tag="wk", name="wk")
        wv_t = wpool.tile([128, NE * 128], F32R, tag="wv", name="wv")

        # ---- psum pools ----
        psp = ctx.enter_context(tc.tile_pool(name="psp", bufs=2, space="PSUM"))
        pssp = ctx.enter_context(tc.tile_pool(name="pssp", bufs=2, space="PSUM"))
        psop = ctx.enter_context(tc.tile_pool(name="psop", bufs=2, space="PSUM"))
        psdp = ctx.enter_context(tc.tile_pool(name="psdp", bufs=2, space="PSUM"))

        # ---- sbuf working pools ----
        xpool = ctx.enter_context(tc.tile_pool(name="xcol", bufs=20))
        qrpool = ctx.enter_context(tc.tile_pool(name="qraw", bufs=6))
        ptpool = ctx.enter_context(tc.tile_pool(name="pt", bufs=4))
        srpool = ctx.enter_context(tc.tile_pool(name="sr", bufs=4))
        ospool = ctx.enter_context(tc.tile_pool(name="osb", bufs=2))

        def rope(dst_ap, ps, cols):
            """dst = raw*cos + swap(raw)*sgn_sin, raw in psum ps [128, TB]."""
            qraw = qrpool.tile([128, TB], F32, tag="qraw", name="qraw")
            nc.scalar.copy(_r(qraw[:]), ps[:])
            ps_sw = pssp.tile([128, TB], F32, tag="pss", name="psw")
            nc.tensor.matmul(ps_sw[:], _r(swp[:]), _r(qraw[:]), start=True, stop=True)
            tmp1 = qrpool.tile([128, TB], F32, tag="qraw", name="ropetmp1")
            tmp2 = qrpool.tile([128, TB], F32, tag="qraw", name="ropetmp2")
            nc.vector.tensor_mul(tmp1[:], qraw[:], cs2[:, cols])
            nc.vector.tensor_mul(tmp2[:], ps_sw[:], snpm[:, cols])
            nc.vector.tensor_add(_r(dst_ap), tmp1[:], tmp2[:])

        # ================= projection phase =================
        for tb in range(NTB):
            cols = slice(tb * TB, (tb + 1) * TB)
            xc = []
            for e in range(NE):
                t_ = xpool.tile([128, TB], F32R, tag="xc", name="xc")
                nc.sync.dma_start(t_[:], _r(ins["xT"][e * 128:(e + 1) * 128, cols]))
                xc.append(t_)
            if tb == 0:
                # weights ordered so PE can start on k while q weights stream
                for e in range(NE):
                    r0 = e * 128
                    nc.sync.dma_start(wk_t[:, e * 128:(e + 1) * 128], _r(ins["wk"][r0:r0 + 128, :]))
                nc.sync.dma_start(swp[:], _r(ins["swp"][:]))
                for e in range(NE):
                    r0 = e * 128
                    nc.sync.dma_start(wv_t[:, e * 128:(e + 1) * 128], _r(ins["wv"][r0:r0 + 128, :]))
                nc.sync.dma_start(iden[:], ins["iden"][:])
                for e in range(NE):
                    r0 = e * 128
                    nc.sync.dma_start(wq_t[:, e * 512:(e + 1) * 512], _r(ins["wq"][r0:r0 + 128, :]))
                nc.sync.dma_start(cs2[:], ins["cs2"][:])
                nc.sync.dma_start(snpm[:], ins["snpm"][:])
                nc.sync.dma_start(tri[:], ins["tri"][:])
                nc.sync.dma_start(ones[:], _r(ins["onec"][:]))
                nc.sync.dma_start(zer[:], ins["zer"][:])

            ps_k = psp.tile([128, TB], F32, tag="ps", name="ps")
            for e in range(NE):
                nc.tensor.matmul(ps_k[:], _r(wk_t[:, e * 128:(e + 1) * 128]),
                                 _r(xc[e][:]), start=(e == 0), stop=(e == NE - 1))
            rope(krot[:, cols], ps_k, cols)

            ps_v = psp.tile([128, TB], F32, tag="ps", name="ps")
            for e in range(NE):
                nc.tensor.matmul(ps_v[:], _r(wv_t[:, e * 128:(e + 1) * 128]),
                                 _r(xc[e][:]), start=(e == 0), stop=(e == NE - 1))
            vtmp = qrpool.tile([128, TB], F32, tag="qraw", name="vtmp")
            nc.scalar.copy(vtmp[:], ps_v[:])

            # transpose v tiles of this block: vtmp [d, s] -> vsd [s, d]
            for jj in range(4):
                j = 4 * tb + jj
                pst = psp.tile([128, 128], F32, tag="ps", name="ps")
                nc.tensor.transpose(pst[:], vtmp[:, jj * 128:(jj + 1) * 128], iden[:])
                nc.scalar.copy(_r(vsd[:, j * 128:(j + 1) * 128]), pst[:])

            for dq in range(HPG):
                ps = psp.tile([128, TB], F32, tag="ps", name="ps")
                for e in range(NE):
                    nc.tensor.matmul(
                        ps[:],
                        _r(wq_t[:, e * 512 + dq * 128: e * 512 + (dq + 1) * 128]),
                        _r(xc[e][:]), start=(e == 0), stop=(e == NE - 1))
                rope(qrot[dq][:, cols], ps, cols)

        # wo: packed [128, 16*512], col block (hh*4+eo)
        wo_t = wpool.tile([128, NE * 512], F32R, tag="wbig", name="wo")
        for hh in range(HPG):
            for eo in range(4):
                blk = hh * 4 + eo
                nc.sync.dma_start(
                    wo_t[:, blk * 512:(blk + 1) * 512],
                    _r(ins["wo"][hh * 128:(hh + 1) * 128, eo * 512:(eo + 1) * 512]))

        # ================= attention phase =================
        for bi in range(NTB):
            for h in range(HPG):
                jmax = 4 * bi + 3
                pso = psop.tile([128, TB], F32, tag="pso", name="pso")
                psd = psdp.tile([128, TB], F32, tag="psd", name="psd")
                for j in range(jmax + 1):
                    diag = (j // 4 == bi)
                    o = 128 * (j - 4 * bi) if diag else 0
                    oe = min(o, 256)
                    W = TB - oe
                    pss = pssp.tile([128, TB], F32, tag="pss", name="pss")
                    nc.tensor.matmul(
                        pss[:, 0:W],
                        _r(krot[:, j * 128:(j + 1) * 128]),
                        _r(qrot[h][:, bi * TB + oe:(bi + 1) * TB]),
                        start=True, stop=True)
                    pt = ptpool.tile([128, TB], F32, tag="pt", name="pt")
                    nc.scalar.activation(_r(pt[:, oe:TB]), pss[:, 0:W], EXP)
                    if diag:
                        if o > 0:
                            nc.vector.tensor_copy(_r(pt[:, 0:o]), zer[:, 0:o])
                        nc.vector.tensor_mul(_r(pt[:, o:o + 128]), pt[:, o:o + 128], tri[:])
                    nc.tensor.matmul(psd[:], _r(ones[:]), _r(pt[:]),
                                     start=(j == 0), stop=(j == jmax))
                    nc.tensor.matmul(pso[:], _r(vsd[:, j * 128:(j + 1) * 128]),
                                     _r(pt[:]), start=(j == 0), stop=(j == jmax))
                cols = slice(bi * TB, (bi + 1) * TB)
                rden = srpool.tile([128, TB], F32, tag="rden", name="rden")
                nc.vector.reciprocal_approx_fast(rden[:], psd[:])
                nc.vector.tensor_mul(_r(aout[h][:, cols]), pso[:], rden[:])

        # ================= output projection =================
        for tq in range(NST):
            trows = slice(tq * 128, (tq + 1) * 128)
            for half in range(2):
                poa = pssp.tile([128, TB], F32, tag="pss", name="pss")
                pob = psop.tile([128, TB], F32, tag="pso", name="pso")
                for hh in range(HPG):
                    lh = _r(aout[hh][:, trows])
                    ba = hh * 4 + 2 * half
                    nc.tensor.matmul(poa[:], lh, _r(wo_t[:, ba * 512:(ba + 1) * 512]),
                                     start=(hh == 0), stop=(hh == HPG - 1))
                    nc.tensor.matmul(pob[:], lh, _r(wo_t[:, (ba + 1) * 512:(ba + 2) * 512]),
                                     start=(hh == 0), stop=(hh == HPG - 1))
                for k, po in ((0, poa), (1, pob)):
                    eo = 2 * half + k
                    osb = ospool.tile([128, TB], F32, tag="osb", name="osb")
                    nc.scalar.copy(osb[:], po[:])
                    nc.sync.dma_start(out_ap[trows, eo * 512:(eo + 1) * 512], osb[:])


# ---------------- host side ----------------

_PERM = np.concatenate([np.arange(0, HD, 2), np.arange(1, HD, 2)])  # half-split


def host_prep(inputs):
    """Full inputs -> list of 8 per-core input dicts (core i = (b=i//4, g=i%4))."""
    x = np.asarray(inputs["x"], dtype=np.float32)
    Wq = np.asarray(inputs["Wq"], dtype=np.float32)
    Wk = np.asarray(inputs["Wk"], dtype=np.float32)
    Wv = np.asarray(inputs["Wv"], dtype=np.float32)
    Wo = np.asarray(inputs["Wo"], dtype=np.float32)

    inv = (10000.0 ** (-np.arange(0, HD, 2, dtype=np.float32) / HD)).astype(np.float32)
    tpos = np.arange(T, dtype=np.float32)
    fr = np.outer(tpos, inv)                       # [T, 64]
    cosT = np.cos(fr).T.astype(np.float32)         # [64, T]
    sinT = np.sin(fr).T.astype(np.float32)
    cs2 = np.concatenate([cosT, cosT], axis=0)     # [128, T]
    snpm = np.concatenate([-sinT, sinT], axis=0)   # [128, T]

    tri = (np.arange(128)[None, :] >= np.arange(128)[:, None]).astype(np.float32)
    swp = np.zeros((128, 128), dtype=np.float32)
    swp[(np.arange(128) + 64) % 128, np.arange(128)] = 1.0
    iden = np.eye(128, dtype=np.float32)

    scale = np.float32(1.0 / np.sqrt(HD))
    xT = [np.ascontiguousarray(x[b].T) for b in range(B)]

    in_maps = []
    for i in range(N_CORES):
        b, g = i // 4, i % 4
        # wq: rows for heads g*4..g*4+3, each permuted, scaled; -> [E, 512]
        rows = []
        for h in range(HPG):
            base = (g * HPG + h) * HD
            rows.append(Wq[base + _PERM, :])
        wq_c = (np.concatenate(rows, axis=0) * scale).T  # [E, 512]
        wk_c = Wk[g * HD + _PERM, :].T                   # [E, 128]
        wv_c = Wv[g * HD:(g + 1) * HD, :].T              # [E, 128]
        wo_c = np.ascontiguousarray(Wo[:, g * 512:(g + 1) * 512].T)  # [512, E]
        in_maps.append({
            "xT": xT[b],
            "wq": np.ascontiguousarray(wq_c),
            "wk": np.ascontiguousarray(wk_c),
            "wv": np.ascontiguousarray(wv_c),
            "wo": wo_c,
            "cs2": cs2, "snpm": snpm, "tri": tri, "swp": swp, "iden": iden,
            "onec": np.ones((128, 128), dtype=np.float32),
            "zer": np.zeros((128, TB), dtype=np.float32),
        })
    return in_maps


_NC = None


def build_nc():
    global _NC
    if _NC is not None:
        return _NC
    nc = bacc.Bacc("TRN2", target_bir_lowering=False, debug=False,
                   num_devices=N_CORES)
    ins = {
        "xT": nc.dram_tensor("xT", [E, T], F32R, kind="ExternalInput").ap(),
        "wq": nc.dram_tensor("wq", [E, HPG * HD], F32R, kind="ExternalInput").ap(),
        "wk": nc.dram_tensor("wk", [E, HD], F32R, kind="ExternalInput").ap(),
        "wv": nc.dram_tensor("wv", [E, HD], F32R, kind="ExternalInput").ap(),
        "wo": nc.dram_tensor("wo", [HPG * HD, E], F32R, kind="ExternalInput").ap(),
        "cs2": nc.dram_tensor("cs2", [128, T], F32, kind="ExternalInput").ap(),
        "snpm": nc.dram_tensor("snpm", [128, T], F32, kind="ExternalInput").ap(),
        "tri": nc.dram_tensor("tri", [128, 128], F32, kind="ExternalInput").ap(),
        "swp": nc.dram_tensor("swp", [128, 128], F32R, kind="ExternalInput").ap(),
        "iden": nc.dram_tensor("iden", [128, 128], F32, kind="ExternalInput").ap(),
        "onec": nc.dram_tensor("onec", [128, 128], F32R, kind="ExternalInput").ap(),
        "zer": nc.dram_tensor("zer", [128, TB], F32, kind="ExternalInput").ap(),
    }
    out = nc.dram_tensor("out", [T, E], F32, kind="ExternalOutput").ap()
    with tile.TileContext(nc) as tc:
        build_body(tc, out, ins)
    nc.compile()
    _NC = nc
    return nc


def gather(results):
    """results: list of 8 dicts with 'out' [T, E] partials -> [B, T, E]."""
    out = np.zeros((B, T, E), dtype=np.float32)
    for i in range(N_CORES):
        out[i // 4] += results[i]["out"]
    return out


def kernel(**inputs):
    nc = build_nc()
    in_maps = host_prep(inputs)
    res = run_bass_kernel_spmd(nc, in_maps, core_ids=list(range(N_CORES)))
    return gather(res.results)


if __name__ == "__main__":
    rng = np.random.default_rng(0)
    ins = {
        "x": rng.standard_normal((B, T, E), dtype=np.float32),
        "Wq": rng.standard_normal((E, E), dtype=np.float32) * 0.02,
        "Wk": rng.standard_normal((NG * HD, E), dtype=np.float32) * 0.02,
        "Wv": rng.standard_normal((NG * HD, E), dtype=np.float32) * 0.02,
        "Wo": rng.standard_normal((E, E), dtype=np.float32) * 0.02,
    }
    out = kernel(**ins)
    print(out.shape, out.dtype, np.abs(out).mean())



# revision 3
# speedup vs baseline: 1.4783x; 1.4783x over previous
"""GroupedQueryAttention Trainium2 kernel (8 NeuronCores), v2.

Sharding: core i handles (batch b = i//4, KV group g = i%4): its 4 query
heads + 1 KV group, full sequence. Each core computes a partial output
(attn_heads @ Wo rows for its heads) in bf16; host sums the 4 partials
per batch in fp32.

v2 layout strategy (per core), all PE inputs bf16 (PSUM accum fp32):
  - projections: W-stationary matmuls over 16 e-chunks, moving x bf16.
  - RoPE: host permutes W rows to half-split layout; swap-half via a
    permutation matmul on PE; raw+swapped copied psum->sbuf bf16 on ACT;
    cos/sin multiplies+add on DVE at 4x bf16 rate.
  - attention: per (bi, h): j-pairs share a [128,1024] psum tile (two
    512-wide score matmuls), one batched exp on ACT per non-diag pair,
    denominators via ones-matmul and PV accumulated into a combined
    [128,1024] psum tile (pso | psd); normalization on DVE.
  - out projection: per 128-row tq tile, 4x 512-col accumulation chains
    into two [128,1024] psum tiles, drained by DVE to bf16, one DMA per
    tq row-block. Interleaved per bi-block right after its attention.
PSUM: exactly 2 pools x [128,1024] f32 x 2 bufs = 8 banks.
"""

import numpy as np
import ml_dtypes
from contextlib import ExitStack

import concourse.bass as bass
import concourse.bacc as bacc
import concourse.tile as tile
import concourse.mybir as mybir
from concourse.bass_utils import run_bass_kernel_spmd

# problem shape (hardcoded per contract)
B, T, E = 2, 2048, 2048
NH, NG, HD = 16, 4, 128
HPG = NH // NG          # 4 heads per group = per core
NE = E // 128           # 16 contraction chunks
TB = 512                # tq / t block
NTB = T // TB           # 4
F32 = mybir.dt.float32
BF16 = mybir.dt.bfloat16
EXP = mybir.ActivationFunctionType.Exp
NPBF16 = ml_dtypes.bfloat16

N_CORES = 8


def build_body(tc, out_ap, ins):
    """ins: dict name -> dram AP. out_ap: [T, E] dram AP (bf16)."""
    nc = tc.nc
    ctx = ExitStack()
    with ctx:
        ctx.enter_context(nc.allow_low_precision(
            reason="bf16 matmul inputs / outputs are intended"))

        # ---- constant / persistent SBUF ----
        const = ctx.enter_context(tc.tile_pool(name="const", bufs=1))
        cs2 = const.tile([128, T], BF16, tag="cs2", name="cs2")
        snpm = const.tile([128, T], BF16, tag="snpm", name="snpm")
        tri = const.tile([128, 128], BF16, tag="tri", name="tri")
        swp = const.tile([128, 128], BF16, tag="swp", name="swp")
        iden = const.tile([128, 128], F32, tag="iden", name="iden")
        ones = const.tile([128, 128], BF16, tag="ones", name="ones")

        persist = ctx.enter_context(tc.tile_pool(name="persist", bufs=1))
        # packed x: [128, tb*8192 + e*512 + c]
        xb = persist.tile([128, NTB * NE * TB], BF16, tag="xb", name="xb")
        qrot = [persist.tile([128, T], BF16, tag=f"qrot{h}", name=f"qrot{h}")
                for h in range(HPG)]
        krot = persist.tile([128, T], BF16, tag="krot", name="krot")
        vsd = persist.tile([128, T], BF16, tag="vsd", name="vsd")
        aout = qrot  # attn output overwrites qrot block-by-block

        # ---- weights (packed by host into sbuf layout) ----
        wpool = ctx.enter_context(tc.tile_pool(name="weights", bufs=1))
        wq_t = wpool.tile([128, NE * 512], BF16, tag="wq", name="wq")
        wk_t = wpool.tile([128, NE * 128], BF16, tag="wk", name="wk")
        wv_t = wpool.tile([128, NE * 128], BF16, tag="wv", name="wv")
        wo_t = wpool.tile([128, NE * 512], BF16, tag="wo", name="wo")

        # ---- psum pools: 2 pools x [128,1024] x 2 bufs = 8 banks ----
        PB = ctx.enter_context(tc.tile_pool(name="pb", bufs=2, space="PSUM"))
        PD = ctx.enter_context(tc.tile_pool(name="pd", bufs=2, space="PSUM"))

        # ---- sbuf working pools ----
        rawp = ctx.enter_context(tc.tile_pool(name="rawp", bufs=3))
        ptp = ctx.enter_context(tc.tile_pool(name="ptp", bufs=4))
        rdp = ctx.enter_context(tc.tile_pool(name="rdp", bufs=2))
        osp = ctx.enter_context(tc.tile_pool(name="osp", bufs=2))

        # ---------------- DMA preload ----------------
        # x for tb0 first (2 halves), then weights in use order, then rest.
        HB = NE * TB // 2  # 4096
        nc.sync.dma_start(xb[:, 0:HB], ins["xb"][:, 0:HB])
        nc.sync.dma_start(xb[:, HB:2 * HB], ins["xb"][:, HB:2 * HB])
        nc.sync.dma_start(wk_t[:], ins["wk"][:])
        nc.sync.dma_start(wv_t[:], ins["wv"][:])
        nc.sync.dma_start(swp[:], ins["swp"][:])
        nc.sync.dma_start(cs2[:], ins["cs2"][:])
        nc.sync.dma_start(snpm[:], ins["snpm"][:])
        nc.sync.dma_start(wq_t[:], ins["wq"][:])
        nc.sync.dma_start(iden[:], ins["iden"][:])
        for tb in range(1, NTB):
            nc.sync.dma_start(xb[:, tb * 2 * HB:(tb + 1) * 2 * HB],
                              ins["xb"][:, tb * 2 * HB:(tb + 1) * 2 * HB])
        nc.sync.dma_start(tri[:], ins["tri"][:])
        nc.sync.dma_start(ones[:], ins["ones"][:])
        nc.sync.dma_start(wo_t[:], ins["wo"][:])

        def xc(tb, e):
            base = tb * NE * TB + e * TB
            return xb[:, base:base + TB]

        def rope(dst_ap, ps, cols, tag):
            """dst = raw*cos + swap(raw)*sgn_sin, raw in psum ps [128, TB]."""
            raw = rawp.tile([128, TB], BF16, tag="raw", name=f"raw{tag}")
            nc.scalar.copy(raw[:], ps)
            psw = PB.tile([128, 1024], F32, tag="b", name=f"psw{tag}")
            nc.tensor.matmul(psw[:, 0:TB], swp[:], raw[:], start=True, stop=True)
            sw = rawp.tile([128, TB], BF16, tag="sw", name=f"sw{tag}")
            nc.scalar.copy(sw[:], psw[:, 0:TB])
            tmp1 = rawp.tile([128, TB], BF16, tag="tmp1", name=f"t1{tag}")
            tmp2 = rawp.tile([128, TB], BF16, tag="tmp2", name=f"t2{tag}")
            nc.vector.tensor_mul(tmp1[:], raw[:], cs2[:, cols])
            nc.vector.tensor_mul(tmp2[:], sw[:], snpm[:, cols])
            nc.vector.tensor_add(dst_ap, tmp1[:], tmp2[:])

        # ================= projection phase =================
        for tb in range(NTB):
            cols = slice(tb * TB, (tb + 1) * TB)
            # k & v chains share one PD tile
            pkv = PD.tile([128, 1024], F32, tag="d", name="pkv")
            for e in range(NE):
                nc.tensor.matmul(pkv[:, 0:TB], wk_t[:, e * 128:(e + 1) * 128],
                                 xc(tb, e), start=(e == 0), stop=(e == NE - 1))
            for e in range(NE):
                nc.tensor.matmul(pkv[:, TB:2 * TB], wv_t[:, e * 128:(e + 1) * 128],
                                 xc(tb, e), start=(e == 0), stop=(e == NE - 1))
            rope(krot[:, cols], pkv[:, 0:TB], cols, "k")
            vtmp = rawp.tile([128, TB], F32, tag="vtmp", name="vtmp")
            nc.scalar.copy(vtmp[:], pkv[:, TB:2 * TB])

            # q chains (pairs share PD tiles)
            for hp in range(2):
                pq = PD.tile([128, 1024], F32, tag="d", name="pq")
                for k in range(2):
                    h = 2 * hp + k
                    for e in range(NE):
                        nc.tensor.matmul(
                            pq[:, k * TB:(k + 1) * TB],
                            wq_t[:, e * 512 + h * 128: e * 512 + (h + 1) * 128],
                            xc(tb, e), start=(e == 0), stop=(e == NE - 1))
                for k in range(2):
                    h = 2 * hp + k
                    rope(qrot[h][:, cols], pq[:, k * TB:(k + 1) * TB], cols, f"q{h}")

            # transpose v tiles of this block: vtmp [d, s] -> vsd [s, d]
            ptv = PD.tile([128, 1024], F32, tag="d", name="ptv")
            for jj in range(4):
                nc.tensor.transpose(ptv[:, jj * 128:(jj + 1) * 128],
                                    vtmp[:, jj * 128:(jj + 1) * 128], iden[:])
            nc.vector.tensor_copy(vsd[:, cols], ptv[:, 0:TB])

        # ================= attention + out-projection =================
        for bi in range(NTB):
            qcols = slice(bi * TB, (bi + 1) * TB)
            for h in range(HPG):
                pa = PD.tile([128, 1024], F32, tag="d", name="pa")  # pso|psd
                njp = 2 * bi + 2
                for jp in range(njp):
                    j0, j1 = 2 * jp, 2 * jp + 1
                    o0 = 128 * (j0 - 4 * bi) if j0 >= 4 * bi else 0
                    o1 = 128 * (j1 - 4 * bi) if j1 >= 4 * bi else 0
                    pb = PB.tile([128, 1024], F32, tag="b", name="pb")
                    nc.tensor.matmul(
                        pb[:, 0:TB - o0],
                        krot[:, j0 * 128:(j0 + 1) * 128],
                        qrot[h][:, bi * TB + o0:(bi + 1) * TB],
                        start=True, stop=True)
                    nc.tensor.matmul(
                        pb[:, TB:2 * TB - o1],
                        krot[:, j1 * 128:(j1 + 1) * 128],
                        qrot[h][:, bi * TB + o1:(bi + 1) * TB],
                        start=True, stop=True)
                    pt = ptp.tile([128, 1024], BF16, tag="pt", name="pt")
                    if o0 == 0 and o1 == 0:
                        nc.scalar.activation(pt[:], pb[:], EXP)
                    else:
                        nc.scalar.activation(pt[:, o0:TB], pb[:, 0:TB - o0], EXP)
                        nc.scalar.activation(pt[:, TB + o1:2 * TB],
                                             pb[:, TB:2 * TB - o1], EXP)
                    for half, j, o in ((0, j0, o0), (1, j1, o1)):
                        if j >= 4 * bi:  # diagonal tile handling
                            c0 = half * TB
                            if o > 0:
                                nc.vector.memset(pt[:, c0:c0 + o], 0.0)
                            nc.vector.tensor_mul(pt[:, c0 + o:c0 + o + 128],
                                                 pt[:, c0 + o:c0 + o + 128], tri[:])
                    last = (jp == njp - 1)
                    nc.tensor.matmul(pa[:, TB:2 * TB], ones[:], pt[:, 0:TB],
                                     start=(jp == 0), stop=False)
                    nc.tensor.matmul(pa[:, TB:2 * TB], ones[:], pt[:, TB:2 * TB],
                                     start=False, stop=last)
                    nc.tensor.matmul(pa[:, 0:TB], vsd[:, j0 * 128:(j0 + 1) * 128],
                                     pt[:, 0:TB], start=(jp == 0), stop=False)
                    nc.tensor.matmul(pa[:, 0:TB], vsd[:, j1 * 128:(j1 + 1) * 128],
                                     pt[:, TB:2 * TB], start=False, stop=last)
                rden = rdp.tile([128, TB], F32, tag="rden", name="rden")
                nc.vector.reciprocal_approx_fast(rden[:], pa[:, TB:2 * TB])
                nc.vector.tensor_mul(aout[h][:, qcols], pa[:, 0:TB], rden[:])

            # out-projection for this bi block
            for tq in range(4):
                trows = slice(bi * TB + tq * 128, bi * TB + (tq + 1) * 128)
                po = [PD.tile([128, 1024], F32, tag="d", name="po") for _ in range(2)]
                for eo in range(4):
                    tgt = po[eo // 2][:, (eo % 2) * TB:(eo % 2 + 1) * TB]
                    for hh in range(HPG):
                        nc.tensor.matmul(
                            tgt, aout[hh][:, trows],
                            wo_t[:, (hh * 4 + eo) * 512:(hh * 4 + eo + 1) * 512],
                            start=(hh == 0), stop=(hh == HPG - 1))
                osb = osp.tile([128, 2048], BF16, tag="osb", name="osb")
                nc.vector.tensor_copy(osb[:, 0:1024], po[0][:])
                nc.vector.tensor_copy(osb[:, 1024:2048], po[1][:])
                nc.sync.dma_start(out_ap[trows, :], osb[:])


# ---------------- host side ----------------

_PERM = np.concatenate([np.arange(0, HD, 2), np.arange(1, HD, 2)])  # half-split


def _pack_w(w):
    """[E, C] -> [128, NE*C] sbuf layout (col block = e-chunk)."""
    c = w.shape[1]
    return np.ascontiguousarray(
        w.reshape(NE, 128, c).transpose(1, 0, 2).reshape(128, NE * c)
    ).astype(NPBF16)


def host_prep(inputs):
    """Full inputs -> list of 8 per-core input dicts (core i = (b=i//4, g=i%4))."""
    x = np.asarray(inputs["x"], dtype=np.float32)
    Wq = np.asarray(inputs["Wq"], dtype=np.float32)
    Wk = np.asarray(inputs["Wk"], dtype=np.float32)
    Wv = np.asarray(inputs["Wv"], dtype=np.float32)
    Wo = np.asarray(inputs["Wo"], dtype=np.float32)

    inv = (10000.0 ** (-np.arange(0, HD, 2, dtype=np.float32) / HD)).astype(np.float32)
    tpos = np.arange(T, dtype=np.float32)
    fr = np.outer(tpos, inv)                       # [T, 64]
    cosT = np.cos(fr).T.astype(np.float32)         # [64, T]
    sinT = np.sin(fr).T.astype(np.float32)
    cs2 = np.concatenate([cosT, cosT], axis=0).astype(NPBF16)     # [128, T]
    snpm = np.concatenate([-sinT, sinT], axis=0).astype(NPBF16)   # [128, T]

    tri = (np.arange(128)[None, :] >= np.arange(128)[:, None]).astype(NPBF16)
    swp = np.zeros((128, 128), dtype=np.float32)
    swp[(np.arange(128) + 64) % 128, np.arange(128)] = 1.0
    swp = swp.astype(NPBF16)
    iden = np.eye(128, dtype=np.float32)
    ones = np.ones((128, 128), dtype=np.float32).astype(NPBF16)

    scale = np.float32(1.0 / np.sqrt(HD))
    # xb[b]: [128, tb*8192 + e*512 + c] = x[b][tb*512+c, e*128+p]
    xbs = []
    for b in range(B):
        xT = x[b].T                                  # [E, T]
        v = xT.reshape(NE, 128, NTB, TB).transpose(1, 2, 0, 3)
        xbs.append(np.ascontiguousarray(v.reshape(128, NTB * NE * TB)).astype(NPBF16))

    in_maps = []
    for i in range(N_CORES):
        b, g = i // 4, i % 4
        rows = []
        for h in range(HPG):
            base = (g * HPG + h) * HD
            rows.append(Wq[base + _PERM, :])
        wq_c = (np.concatenate(rows, axis=0) * scale).T  # [E, 512]
        wk_c = Wk[g * HD + _PERM, :].T                   # [E, 128]
        wv_c = Wv[g * HD:(g + 1) * HD, :].T              # [E, 128]
        # wo blocks (hh, eo): [128, (hh*4+eo)*512 + c] = WoT[hh*128+p, eo*512+c]
        wo_c = Wo[:, g * 512:(g + 1) * 512].T            # [512, E]
        wo_p = np.ascontiguousarray(
            wo_c.reshape(HPG, 128, 4, 512).transpose(1, 0, 2, 3).reshape(128, NE * 512)
        ).astype(NPBF16)
        in_maps.append({
            "xb": xbs[b],
            "wq": _pack_w(wq_c),
            "wk": _pack_w(wk_c),
            "wv": _pack_w(wv_c),
            "wo": wo_p,
            "cs2": cs2, "snpm": snpm, "tri": tri, "swp": swp, "iden": iden,
            "ones": ones,
        })
    return in_maps


_NC = None


def build_nc():
    global _NC
    if _NC is not None:
        return _NC
    nc = bacc.Bacc("TRN2", target_bir_lowering=False, debug=False,
                   num_devices=N_CORES)
    ins = {
        "xb": nc.dram_tensor("xb", [128, NTB * NE * TB], BF16, kind="ExternalInput").ap(),
        "wq": nc.dram_tensor("wq", [128, NE * 512], BF16, kind="ExternalInput").ap(),
        "wk": nc.dram_tensor("wk", [128, NE * 128], BF16, kind="ExternalInput").ap(),
        "wv": nc.dram_tensor("wv", [128, NE * 128], BF16, kind="ExternalInput").ap(),
        "wo": nc.dram_tensor("wo", [128, NE * 512], BF16, kind="ExternalInput").ap(),
        "cs2": nc.dram_tensor("cs2", [128, T], BF16, kind="ExternalInput").ap(),
        "snpm": nc.dram_tensor("snpm", [128, T], BF16, kind="ExternalInput").ap(),
        "tri": nc.dram_tensor("tri", [128, 128], BF16, kind="ExternalInput").ap(),
        "swp": nc.dram_tensor("swp", [128, 128], BF16, kind="ExternalInput").ap(),
        "iden": nc.dram_tensor("iden", [128, 128], F32, kind="ExternalInput").ap(),
        "ones": nc.dram_tensor("ones", [128, 128], BF16, kind="ExternalInput").ap(),
    }
    out = nc.dram_tensor("out", [T, E], BF16, kind="ExternalOutput").ap()
    with tile.TileContext(nc) as tc:
        build_body(tc, out, ins)
    nc.compile()
    _NC = nc
    return nc


def gather(results):
    """results: list of 8 dicts with 'out' [T, E] bf16 partials -> [B, T, E] f32."""
    out = np.zeros((B, T, E), dtype=np.float32)
    for i in range(N_CORES):
        out[i // 4] += np.asarray(results[i]["out"], dtype=np.float32)
    return out


def kernel(**inputs):
    nc = build_nc()
    in_maps = host_prep(inputs)
    res = run_bass_kernel_spmd(nc, in_maps, core_ids=list(range(N_CORES)))
    return gather(res.results)


if __name__ == "__main__":
    rng = np.random.default_rng(0)
    ins = {
        "x": rng.standard_normal((B, T, E), dtype=np.float32),
        "Wq": rng.standard_normal((E, E), dtype=np.float32) * 0.02,
        "Wk": rng.standard_normal((NG * HD, E), dtype=np.float32) * 0.02,
        "Wv": rng.standard_normal((NG * HD, E), dtype=np.float32) * 0.02,
        "Wo": rng.standard_normal((E, E), dtype=np.float32) * 0.02,
    }
    out = kernel(**ins)
    print(out.shape, out.dtype, np.abs(out).mean())
